# revision 10
# baseline (speedup 1.0000x reference)
"""Plastic-RNN step (h = tanh(i2h(x) + hidden @ (w + alpha*hebb)); Hebbian
trace update; linear heads) on 8 Trainium2 NeuronCores.

Sharding: the [H,H] matrices (w, alpha, hebb) are split column-wise into 8
shards of 512 columns. Each core computes its slice of the matvec, the tanh
activation h[:, shard], the hebbian update hebb_new[:, shard], and partial
dot products for the two linear heads. The host concatenates shards and
finishes the (tiny) softmax / bias adds.
"""

import os
import sys
import types

sys.path.insert(0, "/opt/trn_rl_repo")

import numpy as np

H = 4096
NIN = 17
NA = 4
NCORES = 8
S = H // NCORES          # columns per core
P = 128                  # SBUF partitions
CHUNKS = H // P          # 32 row-chunks per shard
GROUP = 4                # chunks per DMA group (1 MiB transfers)
NGROUPS = CHUNKS // GROUP

LAST_EXEC_TIME_NS = None
LAST_RESULTS = None


def _register_ntff_hook():
    """Best-effort registration of the axon NTFF profile hook (the image's
    antenv stub lacks it). Only needed when profiling (BASS_TRACE=1)."""
    try:
        import antenv
        from trn_agent_boot.trn_boot import _ntff_profile_via_ctypes

        if "antenv.axon_hooks" not in sys.modules:
            hook = _ntff_profile_via_ctypes("/opt/axon/libaxon_pjrt.so")
            m = types.ModuleType("antenv.axon_hooks")
            m.get_axon_ntff_profile_hook = lambda: hook
            m.set_axon_ntff_profile_hook = lambda h: None
            sys.modules["antenv.axon_hooks"] = m
            antenv.axon_hooks = m
    except Exception:
        pass


_NC = None


def _build():
    import concourse.bacc as bacc
    import concourse.mybir as mybir
    import concourse.tile as tile

    phases = int(os.environ.get("KERNEL_PHASES", "3"))  # 1=matvec, 2=+head, 3=+hebb

    F32 = mybir.dt.float32
    F32R = mybir.dt.float32r
    MUL = mybir.AluOpType.mult
    ADD = mybir.AluOpType.add

    nc = bacc.Bacc(None, target_bir_lowering=False)

    w_d = nc.dram_tensor("w_s", [H, S], F32R, kind="ExternalInput")
    alpha_d = nc.dram_tensor("alpha_s", [H, S], F32, kind="ExternalInput")
    hebb_d = nc.dram_tensor("hebb_s", [H, S], F32, kind="ExternalInput")
    hpm_d = nc.dram_tensor("hidden_pm", [P, CHUNKS], F32R, kind="ExternalInput")
    hrow_d = nc.dram_tensor("hidden_row", [1, H], F32, kind="ExternalInput")
    xT_d = nc.dram_tensor("xT", [NIN + 1, 1], F32R, kind="ExternalInput")
    i2hwT_d = nc.dram_tensor("i2h_wT_s", [NIN + 1, S], F32R, kind="ExternalInput")
    ones5_d = nc.dram_tensor("ones5", [1, P], F32R, kind="ExternalInput")
    eta_d = nc.dram_tensor("eta2", [1, 1], F32, kind="ExternalInput")
    hv_d = nc.dram_tensor("hv_s", [NA + 1, S], F32, kind="ExternalInput")

    h_o = nc.dram_tensor("h_out", [1, S], F32R, kind="ExternalOutput")
    hebb_o = nc.dram_tensor("hebb_out", [H, S], F32, kind="ExternalOutput")
    pv_o = nc.dram_tensor("pv_out", [NA + 1, 1], F32, kind="ExternalOutput")

    def shard3(d, g):
        return d[g * GROUP * P:(g + 1) * GROUP * P, :].rearrange(
            "(n p) m -> p n m", p=P
        )

    with tile.TileContext(nc) as tc:
        with (
            tc.tile_pool(name="const", bufs=1) as constp,
            tc.tile_pool(name="hebbres", bufs=NGROUPS) as hebbp,
            tc.tile_pool(name="wstream", bufs=2) as wp,
            tc.tile_pool(name="astream", bufs=2) as astp,
            tc.tile_pool(name="ahtmp", bufs=4) as ahp,
            tc.tile_pool(name="outs", bufs=2) as outp,
            tc.tile_pool(name="small", bufs=1) as smallp,
            tc.tile_pool(name="psumh", bufs=1, space="PSUM") as psumh,
            tc.tile_pool(name="psumo", bufs=4, space="PSUM") as psumo,
        ):
            # ---- small inputs ----
            hpm_t = constp.tile([P, CHUNKS], F32R)
            nc.sync.dma_start(hpm_t[:], hpm_d[:])
            hrow_t = constp.tile([1, H], F32)
            nc.sync.dma_start(hrow_t[:], hrow_d[:])
            xT_t = constp.tile([NIN + 1, 1], F32R)
            nc.sync.dma_start(xT_t[:], xT_d[:])
            i2hw_t = constp.tile([NIN + 1, S], F32R)
            nc.sync.dma_start(i2hw_t[:], i2hwT_d[:])
            ones5_t = constp.tile([1, P], F32R)
            nc.sync.dma_start(ones5_t[:], ones5_d[:])
            eta_t = constp.tile([P, 1], F32)
            nc.sync.dma_start(eta_t[:], eta_d[0:1, 0:1].to_broadcast((P, 1)))
            hv_t = constp.tile([NA + 1, S], F32)
            nc.sync.dma_start(hv_t[:], hv_d[:])

            om_eta = constp.tile([P, 1], F32)  # 1 - eta, per partition
            nc.vector.tensor_scalar(om_eta[:], eta_t[:], -1.0, 1.0, MUL, ADD)
            etah_t = constp.tile([1, H], F32R)  # eta * hidden
            nc.vector.tensor_scalar(
                etah_t[:], hrow_t[:], eta_t[0:1, 0:1], None, MUL
            )

            # ---- phase 1: z = x@i2h_w_s.T + i2h_b_s + hidden @ (w+alpha*hebb) ----
            psum_h = psumh.tile([1, S], F32)
            nc.tensor.matmul(psum_h[:], xT_t[:], i2hw_t[:], start=True, stop=False)

            hebb_tiles = []
            for g in range(NGROUPS):
                w_t = wp.tile([P, GROUP, S], F32R)
                a_t = astp.tile([P, GROUP, S], F32)
                hb_t = hebbp.tile([P, GROUP, S], F32)
                hebb_tiles.append(hb_t)
                nc.sync.dma_start(w_t[:], shard3(w_d, g))
                nc.sync.dma_start(a_t[:], shard3(alpha_d, g))
                nc.sync.dma_start(hb_t[:], shard3(hebb_d, g))
                for k in range(GROUP):
                    c = g * GROUP + k
                    ah_t = ahp.tile([P, S], F32R)
                    nc.vector.tensor_mul(ah_t[:], a_t[:, k, :], hb_t[:, k, :])
                    nc.tensor.matmul(
                        psum_h[:], hpm_t[:, c:c + 1], w_t[:, k, :],
                        start=False, stop=False,
                    )
                    nc.tensor.matmul(
                        psum_h[:], hpm_t[:, c:c + 1], ah_t[:],
                        start=False, stop=(c == CHUNKS - 1),
                    )

            # ---- phase 2: h = tanh(z); head partials ----
            h_t = smallp.tile([1, S], F32R)
            nc.scalar.activation(
                h_t[:], psum_h[:], mybir.ActivationFunctionType.Tanh
            )
            nc.sync.dma_start(h_o[:], h_t[:])

            if phases >= 2 and os.environ.get("KERNEL_HEAD", "1") == "1":
                _head(nc, mybir, psumh, smallp, ones5_t, hv_t, h_t, pv_o, MUL, ADD)
            else:
                junk = smallp.tile([NA + 1, 1], F32)
                nc.vector.memset(junk[:], 0.0)
                nc.sync.dma_start(pv_o[:], junk[:])

            # ---- phase 3: hebb_new = (1-eta)*hebb + outer(eta*hidden, h) ----
            for g in range(NGROUPS):
                o_t = outp.tile([P, GROUP, S], F32)
                for k in range(GROUP):
                    c = g * GROUP + k
                    if phases >= 3:
                        ps_o = psumo.tile([P, S], F32)
                        nc.tensor.matmul(
                            ps_o[:], etah_t[0:1, c * P:(c + 1) * P], h_t[:],
                            start=True, stop=True,
                        )
                        nc.vector.scalar_tensor_tensor(
                            o_t[:, k, :], hebb_tiles[g][:, k, :], om_eta[:],
                            ps_o[:], MUL, ADD,
                        )
                    else:
                        nc.vector.tensor_copy(o_t[:, k, :], hebb_tiles[g][:, k, :])
                nc.sync.dma_start(shard3(hebb_o, g), o_t[:])

    nc.compile()
    return nc


def _head(nc, mybir, psumh, smallp, ones5_t, hv_t, h_t, pv_o, MUL, ADD):
    F32 = mybir.dt.float32
    ps5 = psumh.tile([P, S], F32)  # h broadcast to all 128 partitions
    nc.tensor.matmul(ps5[:], ones5_t[:], h_t[:], start=True, stop=True)
    ttr_t = smallp.tile([NA + 1, S], F32)
    pv_t = smallp.tile([NA + 1, 1], F32)
    nc.vector.tensor_mul(ttr_t[:], hv_t[:], ps5[0:NA + 1, :])
    nc.vector.tensor_reduce(pv_t[:], ttr_t[:], mybir.AxisListType.X, ADD)
    nc.sync.dma_start(pv_o[:], pv_t[:])


def _get_nc():
    global _NC
    if _NC is None:
        _register_ntff_hook()
        import concourse.bass_utils as bass_utils

        bass_utils.upload_artifacts = lambda tmpdir: tmpdir  # no object store
        _NC = _build()
    return _NC


def kernel(**inputs):
    global LAST_EXEC_TIME_NS, LAST_RESULTS
    nc = _get_nc()
    from concourse.bass_utils import run_bass_kernel_spmd

    f = np.float32
    x = np.ascontiguousarray(np.asarray(inputs["x"], f))
    hidden = np.ascontiguousarray(np.asarray(inputs["hidden"], f))
    hebb = np.asarray(inputs["hebb"], f)
    i2h_w = np.asarray(inputs["i2h_w"], f)
    i2h_b = np.asarray(inputs["i2h_b"], f)
    w = np.asarray(inputs["w"], f)
    alpha = np.asarray(inputs["alpha"], f)
    eta = np.asarray(inputs["eta"], f)
    h2o_w = np.asarray(inputs["h2o_w"], f)
    h2o_b = np.asarray(inputs["h2o_b"], f)
    h2v_w = np.asarray(inputs["h2v_w"], f)
    h2v_b = np.asarray(inputs["h2v_b"], f)

    hpm = np.ascontiguousarray(hidden.reshape(CHUNKS, P).T)
    xT = np.ascontiguousarray(np.concatenate([x.T, np.ones((1, 1), f)], 0))
    ones5 = np.ones((1, P), f)
    eta2 = np.ascontiguousarray(eta.reshape(1, 1))

    in_maps = []
    for c in range(NCORES):
        j0 = c * S
        in_maps.append(
            {
                "w_s": np.ascontiguousarray(w[:, j0:j0 + S]),
                "alpha_s": np.ascontiguousarray(alpha[:, j0:j0 + S]),
                "hebb_s": np.ascontiguousarray(hebb[:, j0:j0 + S]),
                "hidden_pm": hpm,
                "hidden_row": hidden,
                "xT": xT,
                "i2h_wT_s": np.ascontiguousarray(
                    np.concatenate(
                        [i2h_w[j0:j0 + S, :].T, i2h_b[j0:j0 + S][None, :]], 0
                    )
                ),
                "ones5": ones5,
                "eta2": eta2,
                "hv_s": np.ascontiguousarray(
                    np.concatenate([h2o_w[:, j0:j0 + S], h2v_w[:, j0:j0 + S]], 0)
                ),
            }
        )

    res = run_bass_kernel_spmd(nc, in_maps, core_ids=list(range(NCORES)))
    LAST_EXEC_TIME_NS = res.exec_time_ns
    LAST_RESULTS = res

    h = np.concatenate([r["h_out"] for r in res.results], axis=1)
    hebb_new = np.concatenate([r["hebb_out"] for r in res.results], axis=1)
    pv = np.stack([r["pv_out"][:, 0] for r in res.results]).sum(axis=0)

    logits = pv[:NA] + h2o_b
    zmax = logits.max()
    ez = np.exp(logits - zmax)
    activout = (ez / ez.sum())[None, :].astype(f)
    valueout = np.array([[pv[NA] + h2v_b[0]]], f)
    return activout, valueout, h.astype(f), hebb_new.astype(f)


# revision 11
# speedup vs baseline: 1.4576x; 1.4576x over previous
"""Plastic-RNN step (h = tanh(i2h(x) + hidden @ (w + alpha*hebb)); Hebbian
trace update; linear heads) on 8 Trainium2 NeuronCores.

Sharding: the [H,H] matrices (w, alpha, hebb) are split column-wise into 8
shards of 512 columns. Each core computes its slice of the matvec, the tanh
activation h[:, shard], the hebbian update hebb_new[:, shard], and partial
dot products for the two linear heads. The host concatenates shards and
finishes the (tiny) softmax / bias adds.
"""

import os
import sys
import types

sys.path.insert(0, "/opt/trn_rl_repo")

import numpy as np

H = 4096
NIN = 17
NA = 4
NCORES = 8
S = H // NCORES          # columns per core
P = 128                  # SBUF partitions
CHUNKS = H // P          # 32 row-chunks per shard
GROUP = 4                # chunks per DMA group (1 MiB transfers)
NGROUPS = CHUNKS // GROUP

LAST_EXEC_TIME_NS = None
LAST_RESULTS = None


def _register_ntff_hook():
    """Best-effort registration of the axon NTFF profile hook (the image's
    antenv stub lacks it). Only needed when profiling (BASS_TRACE=1)."""
    try:
        import antenv
        from trn_agent_boot.trn_boot import _ntff_profile_via_ctypes

        if "antenv.axon_hooks" not in sys.modules:
            hook = _ntff_profile_via_ctypes("/opt/axon/libaxon_pjrt.so")
            m = types.ModuleType("antenv.axon_hooks")
            m.get_axon_ntff_profile_hook = lambda: hook
            m.set_axon_ntff_profile_hook = lambda h: None
            sys.modules["antenv.axon_hooks"] = m
            antenv.axon_hooks = m
    except Exception:
        pass


_NC = None


def _build():
    import concourse.bacc as bacc
    import concourse.mybir as mybir
    import concourse.tile as tile

    phases = int(os.environ.get("KERNEL_PHASES", "3"))  # 1=matvec, 2=+head, 3=+hebb

    F32 = mybir.dt.float32
    F32R = mybir.dt.float32r
    MUL = mybir.AluOpType.mult
    ADD = mybir.AluOpType.add

    nc = bacc.Bacc(None, target_bir_lowering=False)

    BF16 = mybir.dt.bfloat16
    w_d = nc.dram_tensor("w_s", [H, S], BF16, kind="ExternalInput")
    alpha_d = nc.dram_tensor("alpha_s", [H, S], BF16, kind="ExternalInput")
    hebb_d = nc.dram_tensor("hebb_s", [H, S], F32, kind="ExternalInput")
    hpm_d = nc.dram_tensor("hidden_pm", [P, CHUNKS], BF16, kind="ExternalInput")
    hrow_d = nc.dram_tensor("hidden_row", [1, H], F32, kind="ExternalInput")
    xT_d = nc.dram_tensor("xT", [NIN + 1, 1], F32R, kind="ExternalInput")
    i2hwT_d = nc.dram_tensor("i2h_wT_s", [NIN + 1, S], F32R, kind="ExternalInput")
    ones5_d = nc.dram_tensor("ones5", [1, P], F32R, kind="ExternalInput")
    eta_d = nc.dram_tensor("eta2", [P, 1], F32, kind="ExternalInput")
    hv_d = nc.dram_tensor("hv_s", [NA + 1, S], F32, kind="ExternalInput")

    h_o = nc.dram_tensor("h_out", [1, S], F32R, kind="ExternalOutput")
    hebb_o = nc.dram_tensor("hebb_out", [H, S], F32, kind="ExternalOutput")
    pv_o = nc.dram_tensor("pv_out", [NA + 1, 1], F32, kind="ExternalOutput")

    def shard3(d, g):
        return d[g * GROUP * P:(g + 1) * GROUP * P, :].rearrange(
            "(n p) m -> p n m", p=P
        )

    with tile.TileContext(nc) as tc:
        with (
            tc.tile_pool(name="const", bufs=1) as constp,
            tc.tile_pool(name="hebbres", bufs=NGROUPS) as hebbp,
            tc.tile_pool(name="wstream", bufs=3) as wp,
            tc.tile_pool(name="astream", bufs=3) as astp,
            tc.tile_pool(name="ahtmp", bufs=6) as ahp,
            tc.tile_pool(name="outs", bufs=4) as outp,
            tc.tile_pool(name="small", bufs=1) as smallp,
            tc.tile_pool(name="psumh", bufs=1, space="PSUM") as psumh,
            tc.tile_pool(name="psumo", bufs=4, space="PSUM") as psumo,
        ):
            # ---- small inputs ----
            hpm_t = constp.tile([P, CHUNKS], BF16)
            nc.scalar.dma_start(hpm_t[:], hpm_d[:])
            hrow_t = constp.tile([1, H], F32)
            nc.scalar.dma_start(hrow_t[:], hrow_d[:])
            xT_t = constp.tile([NIN + 1, 1], F32R)
            nc.scalar.dma_start(xT_t[:], xT_d[:])
            i2hw_t = constp.tile([NIN + 1, S], F32R)
            nc.scalar.dma_start(i2hw_t[:], i2hwT_d[:])
            ones5_t = constp.tile([1, P], F32R)
            nc.scalar.dma_start(ones5_t[:], ones5_d[:])
            eta_t = constp.tile([P, 1], F32)
            nc.scalar.dma_start(eta_t[:], eta_d[:])
            hv_t = constp.tile([NA + 1, S], F32)
            nc.scalar.dma_start(hv_t[:], hv_d[:])

            om_eta = constp.tile([P, 1], F32)  # 1 - eta, per partition
            nc.vector.tensor_scalar(om_eta[:], eta_t[:], -1.0, 1.0, MUL, ADD)
            etah_t = constp.tile([1, H], F32R)  # eta * hidden
            nc.vector.tensor_scalar(
                etah_t[:], hrow_t[:], eta_t[0:1, 0:1], None, MUL
            )

            # ---- phase 1: z = x@i2h_w_s.T + i2h_b_s + hidden @ (w+alpha*hebb) ----
            psum_h = psumh.tile([1, S], F32)
            nc.tensor.matmul(psum_h[:], xT_t[:], i2hw_t[:], start=True, stop=False)

            hebb_tiles = []
            for g in range(NGROUPS):
                w_t = wp.tile([P, GROUP, S], BF16)
                a_t = astp.tile([P, GROUP, S], BF16)
                hb_t = hebbp.tile([P, GROUP, S], F32)
                hebb_tiles.append(hb_t)
                nc.sync.dma_start(w_t[:], shard3(w_d, g))
                nc.sync.dma_start(a_t[:], shard3(alpha_d, g))
                nc.sync.dma_start(hb_t[:], shard3(hebb_d, g))
                for k in range(GROUP):
                    c = g * GROUP + k
                    ah_t = ahp.tile([P, S], BF16)
                    nc.vector.tensor_mul(ah_t[:], a_t[:, k, :], hb_t[:, k, :])
                    nc.tensor.matmul(
                        psum_h[:], hpm_t[:, c:c + 1], w_t[:, k, :],
                        start=False, stop=False,
                    )
                    nc.tensor.matmul(
                        psum_h[:], hpm_t[:, c:c + 1], ah_t[:],
                        start=False, stop=(c == CHUNKS - 1),
                    )

            # ---- phase 2: h = tanh(z); head partials ----
            h_t = smallp.tile([1, S], F32R)
            nc.scalar.activation(
                h_t[:], psum_h[:], mybir.ActivationFunctionType.Tanh
            )
            nc.sync.dma_start(h_o[:], h_t[:])

            if phases >= 2 and os.environ.get("KERNEL_HEAD", "1") == "1":
                _head(nc, mybir, psumh, smallp, ones5_t, hv_t, h_t, pv_o, MUL, ADD)
            else:
                junk = smallp.tile([NA + 1, 1], F32)
                nc.vector.memset(junk[:], 0.0)
                nc.sync.dma_start(pv_o[:], junk[:])

            # ---- phase 3: hebb_new = (1-eta)*hebb + outer(eta*hidden, h) ----
            OG = 2
            for g in range(CHUNKS // OG):
                o_t = outp.tile([P, OG, S], F32)
                for k in range(OG):
                    c = g * OG + k
                    if phases >= 3:
                        ps_o = psumo.tile([P, S], F32)
                        nc.tensor.matmul(
                            ps_o[:], etah_t[0:1, c * P:(c + 1) * P], h_t[:],
                            start=True, stop=True,
                        )
                        nc.vector.scalar_tensor_tensor(
                            o_t[:, k, :], hebb_tiles[c // GROUP][:, c % GROUP, :],
                            om_eta[:], ps_o[:], MUL, ADD,
                        )
                    else:
                        nc.vector.tensor_copy(
                            o_t[:, k, :], hebb_tiles[c // GROUP][:, c % GROUP, :]
                        )
                nc.sync.dma_start(
                    hebb_o[g * OG * P:(g + 1) * OG * P, :].rearrange(
                        "(n p) m -> p n m", p=P
                    ),
                    o_t[:],
                )

    nc.compile()
    return nc


def _head(nc, mybir, psumh, smallp, ones5_t, hv_t, h_t, pv_o, MUL, ADD):
    F32 = mybir.dt.float32
    ps5 = psumh.tile([P, S], F32)  # h broadcast to all 128 partitions
    nc.tensor.matmul(ps5[:], ones5_t[:], h_t[:], start=True, stop=True)
    ttr_t = smallp.tile([NA + 1, S], F32)
    pv_t = smallp.tile([NA + 1, 1], F32)
    nc.vector.tensor_mul(ttr_t[:], hv_t[:], ps5[0:NA + 1, :])
    nc.vector.tensor_reduce(pv_t[:], ttr_t[:], mybir.AxisListType.X, ADD)
    nc.sync.dma_start(pv_o[:], pv_t[:])


def _get_nc():
    global _NC
    if _NC is None:
        _register_ntff_hook()
        import concourse.bass_utils as bass_utils

        bass_utils.upload_artifacts = lambda tmpdir: tmpdir  # no object store
        _NC = _build()
    return _NC


def kernel(**inputs):
    global LAST_EXEC_TIME_NS, LAST_RESULTS
    nc = _get_nc()
    from concourse.bass_utils import run_bass_kernel_spmd

    f = np.float32
    x = np.ascontiguousarray(np.asarray(inputs["x"], f))
    hidden = np.ascontiguousarray(np.asarray(inputs["hidden"], f))
    hebb = np.asarray(inputs["hebb"], f)
    i2h_w = np.asarray(inputs["i2h_w"], f)
    i2h_b = np.asarray(inputs["i2h_b"], f)
    w = np.asarray(inputs["w"], f)
    alpha = np.asarray(inputs["alpha"], f)
    eta = np.asarray(inputs["eta"], f)
    h2o_w = np.asarray(inputs["h2o_w"], f)
    h2o_b = np.asarray(inputs["h2o_b"], f)
    h2v_w = np.asarray(inputs["h2v_w"], f)
    h2v_b = np.asarray(inputs["h2v_b"], f)

    import ml_dtypes

    bf16 = ml_dtypes.bfloat16
    hpm = np.ascontiguousarray(hidden.reshape(CHUNKS, P).T.astype(bf16))
    w_bf = w.astype(bf16)
    alpha_bf = alpha.astype(bf16)
    xT = np.ascontiguousarray(np.concatenate([x.T, np.ones((1, 1), f)], 0))
    ones5 = np.ones((1, P), f)
    eta2 = np.ascontiguousarray(np.broadcast_to(eta.reshape(1, 1), (P, 1)))

    in_maps = []
    for c in range(NCORES):
        j0 = c * S
        in_maps.append(
            {
                "w_s": np.ascontiguousarray(w_bf[:, j0:j0 + S]),
                "alpha_s": np.ascontiguousarray(alpha_bf[:, j0:j0 + S]),
                "hebb_s": np.ascontiguousarray(hebb[:, j0:j0 + S]),
                "hidden_pm": hpm,
                "hidden_row": hidden,
                "xT": xT,
                "i2h_wT_s": np.ascontiguousarray(
                    np.concatenate(
                        [i2h_w[j0:j0 + S, :].T, i2h_b[j0:j0 + S][None, :]], 0
                    )
                ),
                "ones5": ones5,
                "eta2": eta2,
                "hv_s": np.ascontiguousarray(
                    np.concatenate([h2o_w[:, j0:j0 + S], h2v_w[:, j0:j0 + S]], 0)
                ),
            }
        )

    res = run_bass_kernel_spmd(nc, in_maps, core_ids=list(range(NCORES)))
    LAST_EXEC_TIME_NS = res.exec_time_ns
    LAST_RESULTS = res

    h = np.concatenate([r["h_out"] for r in res.results], axis=1)
    hebb_new = np.concatenate([r["hebb_out"] for r in res.results], axis=1)
    pv = np.stack([r["pv_out"][:, 0] for r in res.results]).sum(axis=0)

    logits = pv[:NA] + h2o_b
    zmax = logits.max()
    ez = np.exp(logits - zmax)
    activout = (ez / ez.sum())[None, :].astype(f)
    valueout = np.array([[pv[NA] + h2v_b[0]]], f)
    return activout, valueout, h.astype(f), hebb_new.astype(f)
